# revision 17
# baseline (speedup 1.0000x reference)
"""Trainium2 Bass kernel for nn_BackboneModule (AlphaFold-style backbone frames).

Strategy (data-parallel over residues, 8 cores):
  - Host: shard residues 8 ways; within each shard, stable-sort residues by
    residue_type and assign each type a fixed set of SBUF partitions
    (~6 partitions x 588 slots each).  Type is constant per partition, so all
    per-type table values (transforms_tensor / deps / rigids) become true
    per-partition scalars for tensor_scalar / scalar_tensor_tensor FMAs, or
    per-type matmul weights.  The device kernel does NO data-dependent
    gathers.
  - Device, per 98-column chunk:
      * dependency-chain over the 8 rigid groups on DVE/GPSIMD via fused
        FMAs with per-partition constants,
      * parent-frame selection (acc[t_dep[i]]) via SBUF->SBUF DMA copies
        (one per (type, step); parent index known per type),
      * per-atom frame gather + rotation fused into one per-type [96 -> 72]
        float32r matmul on the PE over partition-strided columns of the
        PE-transposed acc planes.
  - Outputs written in kernel-native order; host scatters back.
"""

import numpy as np
import sys, time
sys.path.insert(0, "/opt/trn_rl_repo")

import concourse.bass as bass
import concourse.bacc as bacc
import concourse.tile as tile
from concourse import mybir
from concourse.bass_utils import run_bass_kernel_spmd

N = 524288
N_RESTYPE = 21
MAX_RIGID = 8
MAX_ATOM = 24
N_CORES = 8
SHARD = N // N_CORES          # 65536
P = 128
FTOT = 588                    # slots per partition (128*588 = 75264 >= 65536)
FCH = 98                      # chunk width
N_CHUNKS = FTOT // FCH        # 6
FSUB = 49                     # atom-stage sub-block (per chunk: 2)
FPT = 7                       # transposes per PSUM group

F32 = mybir.dt.float32
F32R = mybir.dt.float32r

_cached = {}
last_results = None


def _plan(residue_type):
    rt = np.asarray(residue_type).astype(np.int32).reshape(N)
    orders, counts = [], np.zeros((N_CORES, N_RESTYPE), np.int64)
    for c in range(N_CORES):
        seg = rt[c * SHARD:(c + 1) * SHARD]
        orders.append(np.argsort(seg, kind="stable"))
        counts[c] = np.bincount(seg, minlength=N_RESTYPE)
    cmax = counts.max(0)
    parts = np.maximum(1, np.ceil(cmax / FTOT).astype(np.int64))
    assert parts.sum() <= P, f"type partition allocation overflow: {parts}"
    pstart = np.zeros(N_RESTYPE + 1, np.int64)
    pstart[1:] = np.cumsum(parts)
    return rt, orders, counts, parts, pstart


def prepare(residue_type, bb, pos0, sc,
            transforms_tensor, rigids_tensor, transforms_dep, rigids_dep):
    """Host prep: returns (nc, in_maps, meta) for the SPMD run."""
    rt, orders, counts, parts, pstart = _plan(residue_type)
    bb = np.asarray(bb, dtype=np.float32).reshape(N, 12)
    pos0 = np.asarray(pos0, dtype=np.float32).reshape(N, 3)
    sc = np.asarray(sc, dtype=np.float32).reshape(N, 14)
    T = np.asarray(transforms_tensor, dtype=np.float32)
    RG = np.asarray(rigids_tensor, dtype=np.float32)
    TD = np.asarray(transforms_dep).astype(np.int32)
    RD = np.asarray(rigids_dep).astype(np.int32)

    # type of each partition (pads -> type 0; their outputs are never read)
    type_of_p = np.zeros(P, np.int64)
    for t in range(N_RESTYPE):
        type_of_p[pstart[t]:pstart[t + 1]] = t

    # atom map W: out72 = W.T @ acc96
    W = np.zeros((N_RESTYPE, 96, 72), np.float32)
    for t in range(N_RESTYPE):
        for a in range(MAX_ATOM):
            g = RD[t, a]
            for i in range(3):
                for j in range(3):
                    W[t, g * 12 + i * 3 + j, a * 3 + i] = RG[t, a, j]
                W[t, g * 12 + 9 + i, a * 3 + i] = 1.0
    W_flat = np.ascontiguousarray(W.transpose(1, 0, 2).reshape(96, N_RESTYPE * 72))

    # consts [128, 96]: col layout c = 12*i + (r*3+k | 9+k), per-partition type
    cval = np.zeros((N_RESTYPE, 96), np.float32)
    for t in range(N_RESTYPE):
        v = []
        for i in range(MAX_RIGID):
            v.extend(T[t, i, :3, :].reshape(9))
            v.extend(T[t, i, 3, :])
        cval[t] = v
    consts = np.ascontiguousarray(cval[type_of_p])       # [128, 96]
    ident = np.eye(P, dtype=np.float32)

    in_maps = []
    for c in range(N_CORES):
        s = slice(c * SHARD, (c + 1) * SHARD)
        cs = sc[s].reshape(-1, 7, 2)
        arrs = np.concatenate([bb[s].T, pos0[s].T, cs[:, :, 0].T, cs[:, :, 1].T], 0)  # [29, SHARD]
        planes = np.zeros((29, P, FTOT), np.float32)
        base = 0
        for t in range(N_RESTYPE):
            cnt = int(counts[c][t])
            idx = orders[c][base:base + cnt]
            base += cnt
            k = np.arange(cnt)
            planes[:, pstart[t] + k // FTOT, k % FTOT] = arrs[:, idx]
        m = {f"pl{q}": np.ascontiguousarray(planes[q]) for q in range(29)}
        m["consts"] = consts
        m["wtab"] = W_flat
        m["ident"] = ident
        in_maps.append(m)

    key = ("prog", TD.tobytes(), parts.tobytes())
    if key not in _cached:
        _cached[key] = _build_program(TD, parts, pstart)
    nc = _cached[key]
    return nc, in_maps, (orders, counts, parts, pstart)


def kernel(residue_type, bb, pos0, sc,
           transforms_tensor, rigids_tensor, transforms_dep, rigids_dep):
    nc, in_maps, (orders, counts, parts, pstart) = prepare(
        residue_type, bb, pos0, sc, transforms_tensor, rigids_tensor,
        transforms_dep, rigids_dep)

    res = run_bass_kernel_spmd(nc, in_maps, core_ids=list(range(N_CORES)))
    global last_results
    last_results = res

    # outR col order: (chunk, sub, tau, flocal, u) with tau-base offsets
    colbase = np.zeros(N_RESTYPE, np.int64)
    o = 0
    for t in range(N_RESTYPE):
        colbase[t] = o
        o += FSUB * int(parts[t])
    span = o                                    # cols per (chunk, sub)

    Rout = np.zeros((N, MAX_ATOM, 3), np.float32)
    Aout = np.zeros((N, 4, 3), np.float32)
    for c in range(N_CORES):
        outR = res.results[c]["outR"]           # [72, N_CHUNKS*2*span]
        outA = res.results[c]["outA"]           # [128, FTOT*12]
        base = 0
        for t in range(N_RESTYPE):
            cnt = int(counts[c][t])
            idx = orders[c][base:base + cnt] + c * SHARD
            base += cnt
            k = np.arange(cnt)
            p, f = k // FTOT, k % FTOT          # partition offset, slot
            chunk, fc = f // FCH, f % FCH
            sub, fl = fc // FSUB, fc % FSUB
            col = (chunk * 2 + sub) * span + colbase[t] + fl * int(parts[t]) + p
            Rout[idx] = outR[:, col].T.reshape(cnt, MAX_ATOM, 3)
            Aout[idx] = outA[(pstart[t] + p)[:, None], (f * 12)[:, None] +
                             np.arange(12)[None, :]].reshape(cnt, 4, 3)
    return Rout, Aout


def _build_program(TD, parts, pstart):
    GSZ = FCH * 12
    nc = bacc.Bacc("TRN2", target_bir_lowering=False, debug=False,
                   num_devices=N_CORES)

    d_in = {q: nc.dram_tensor(f"pl{q}", [P, FTOT], F32, kind="ExternalInput")
            for q in range(29)}
    d_consts = nc.dram_tensor("consts", [P, 96], F32, kind="ExternalInput")
    d_w = nc.dram_tensor("wtab", [96, N_RESTYPE * 72], F32,
                         kind="ExternalInput")
    d_id = nc.dram_tensor("ident", [P, P], F32, kind="ExternalInput")
    span = FSUB * int(parts.sum())
    d_outR = nc.dram_tensor("outR", [72, N_CHUNKS * 2 * span], F32,
                            kind="ExternalOutput")
    d_outA = nc.dram_tensor("outA", [P, FTOT * 12], F32,
                            kind="ExternalOutput")

    mul, add = mybir.AluOpType.mult, mybir.AluOpType.add

    with tile.TileContext(nc) as tc:
        with (tc.tile_pool(name="cpool", bufs=1) as cpool,
              tc.tile_pool(name="iopool", bufs=2) as iopool,
              tc.tile_pool(name="wpool", bufs=2) as wpool,
              tc.tile_pool(name="ppool", bufs=2, space="PSUM") as ppool,
              tc.tile_pool(name="spool", bufs=2) as spool):

            t_consts = cpool.tile([P, 96], F32, name="tconsts")
            nc.sync.dma_start(t_consts[:, :], d_consts.ap())
            t_w = cpool.tile([96, N_RESTYPE * 72], F32, name="tw")
            nc.sync.dma_start(t_w[:, :], d_w.ap())
            t_id = cpool.tile([P, P], F32, name="tid")
            nc.sync.dma_start(t_id[:, :], d_id.ap())

            def cst(cid):
                return t_consts[:, cid:cid + 1]

            for chunk in range(N_CHUNKS):
                col0 = chunk * FCH

                pl = []
                for q in range(29):
                    t = iopool.tile([P, FCH], F32, tag=f"in{q}", name=f"in{q}")
                    nc.sync.dma_start(t[:, :], d_in[q].ap()[:, col0:col0 + FCH])
                    pl.append(t)
                BB, P0 = pl[:12], pl[12:15]
                C, S = pl[15:22], pl[22:29]

                acc = wpool.tile([P, MAX_RIGID * GSZ], F32, tag="acc",
                                 name="acc")

                def acc_e(g, e):
                    return bass.AP(acc.tensor, g * GSZ + e,
                                   [list(acc.ap[0]), [12, FCH]])

                # ---- group 0 ----
                for i in range(3):
                    for j in range(3):
                        o = acc_e(0, i * 3 + j)
                        nc.vector.tensor_scalar(o, BB[j][:, :],
                                                cst(i * 3), None, mul)
                        for k in (1, 2):
                            nc.vector.scalar_tensor_tensor(
                                o, BB[k * 3 + j][:, :], cst(i * 3 + k),
                                o, mul, add)
                pre = []
                for k in range(3):
                    pk = wpool.tile([P, FCH], F32, tag="pre", bufs=4,
                                    name=f"pre{k}")
                    nc.vector.tensor_add(pk[:, :], BB[9 + k][:, :],
                                         P0[k][:, :])
                    pre.append(pk)
                for i in range(3):
                    o = acc_e(0, 9 + i)
                    nc.vector.tensor_scalar(o, pre[0][:, :], cst(i * 3),
                                            cst(9 + i), mul, add)
                    for k in (1, 2):
                        nc.vector.scalar_tensor_tensor(
                            o, pre[k][:, :], cst(i * 3 + k), o, mul, add)

                # ---- steps 1..7 ----
                for i in range(1, MAX_RIGID):
                    prev = wpool.tile([P, GSZ], F32, tag="prev", bufs=2,
                                      name=f"prev{i}")

                    def prev_e(e):
                        return bass.AP(prev.tensor, e,
                                       [list(prev.ap[0]), [12, FCH]])

                    for t in range(N_RESTYPE):
                        d = int(TD[t, i])
                        p0, p1 = int(pstart[t]), int(pstart[t + 1])
                        if t == N_RESTYPE - 1:
                            p1 = P          # cover pad partitions too
                        nc.sync.dma_start(
                            prev[p0:p1, :],
                            acc[p0:p1, d * GSZ:(d + 1) * GSZ])

                    cb = 12 * i
                    for r in range(3):
                        o = acc_e(i, r * 3)
                        nc.vector.tensor_scalar(o, prev_e(r * 3), cst(cb),
                                                None, mul)
                        for j in (1, 2):
                            nc.vector.scalar_tensor_tensor(
                                o, prev_e(r * 3 + j), cst(cb + j * 3),
                                o, mul, add)
                        t1 = wpool.tile([P, FCH], F32, tag="t1", bufs=3,
                                        name=f"t1_{i}_{r}")
                        t2 = wpool.tile([P, FCH], F32, tag="t2", bufs=3,
                                        name=f"t2_{i}_{r}")
                        nc.vector.tensor_scalar(t1[:, :], prev_e(r * 3),
                                                cst(cb + 1), None, mul)
                        for j in (1, 2):
                            nc.vector.scalar_tensor_tensor(
                                t1[:, :], prev_e(r * 3 + j),
                                cst(cb + j * 3 + 1), t1[:, :], mul, add)
                        nc.vector.tensor_scalar(t2[:, :], prev_e(r * 3),
                                                cst(cb + 2), None, mul)
                        for j in (1, 2):
                            nc.vector.scalar_tensor_tensor(
                                t2[:, :], prev_e(r * 3 + j),
                                cst(cb + j * 3 + 2), t2[:, :], mul, add)
                        o1, o2 = acc_e(i, r * 3 + 1), acc_e(i, r * 3 + 2)
                        t3 = wpool.tile([P, FCH], F32, tag="t3", bufs=3,
                                        name=f"t3_{i}_{r}")
                        t4 = wpool.tile([P, FCH], F32, tag="t4", bufs=3,
                                        name=f"t4_{i}_{r}")
                        nc.gpsimd.tensor_mul(t4[:, :], C[i - 1][:, :],
                                             t1[:, :])
                        nc.gpsimd.tensor_mul(t3[:, :], S[i - 1][:, :],
                                             t2[:, :])
                        nc.vector.tensor_add(o1, t4[:, :], t3[:, :])
                        nc.gpsimd.tensor_mul(t4[:, :], C[i - 1][:, :],
                                             t2[:, :])
                        nc.gpsimd.tensor_mul(t3[:, :], S[i - 1][:, :],
                                             t1[:, :])
                        nc.vector.tensor_sub(o2, t4[:, :], t3[:, :])
                    for r in range(3):
                        o = acc_e(i, 9 + r)
                        nc.vector.scalar_tensor_tensor(
                            o, prev_e(r * 3), cst(cb + 9), prev_e(9 + r),
                            mul, add)
                        for j in (1, 2):
                            nc.vector.scalar_tensor_tensor(
                                o, prev_e(r * 3 + j), cst(cb + 9 + j),
                                o, mul, add)

                # ---- store opr0 ----
                nc.sync.dma_start(
                    d_outA.ap()[:, col0 * 12:(col0 + FCH) * 12],
                    acc[:, 0:GSZ])

                # ---- atom stage ----
                for sub in range(FCH // FSUB):
                    asb = spool.tile([96, FSUB * P], F32, tag="asb",
                                     name="asb")
                    for pg in range(FSUB // FPT):
                        f0 = sub * FSUB + pg * FPT
                        stg = wpool.tile([P, FPT * 96], F32, tag="stg",
                                         bufs=3, name="stg")
                        gsrc = bass.AP(acc.tensor, f0 * 12,
                                       [list(acc.ap[0]), [12, FPT],
                                        [GSZ, MAX_RIGID], [1, 12]])
                        nc.scalar.copy(stg[:, :], gsrc)
                        pt = ppool.tile([96, FPT * P], F32, tag="pt",
                                        name="pt")
                        for u in range(FPT):
                            nc.tensor.transpose(
                                pt[:, u * P:(u + 1) * P],
                                stg[:, u * 96:(u + 1) * 96], t_id[:, :])
                        nc.scalar.copy(
                            asb[:, pg * FPT * P:(pg + 1) * FPT * P],
                            pt[:, :])
                    ocol = 0
                    for t in range(N_RESTYPE):
                        pcnt = int(parts[t])
                        ncols = FSUB * pcnt
                        po = ppool.tile([72, 512], F32, tag="po", name="po")
                        rhs = bass.AP(asb.tensor, int(pstart[t]),
                                      [list(asb.ap[0]), [P, FSUB], [1, pcnt]])
                        nc.tensor.matmul(
                            po[:, :ncols], t_w[:, t * 72:(t + 1) * 72],
                            rhs, start=True, stop=True)
                        osb = spool.tile([72, 512], F32, tag="osb",
                                         bufs=3, name="osb")
                        nc.scalar.copy(osb[:, :ncols], po[:, :ncols])
                        gcol = (chunk * 2 + sub) * span + ocol
                        nc.sync.dma_start(
                            d_outR.ap()[:, gcol:gcol + ncols],
                            osb[:, :ncols])
                        ocol += ncols
    nc.finalize()
    return nc


if __name__ == "__main__":
    sys.path.insert(0, "/root/problem")
    import reference
    inputs = {k: np.asarray(v) for k, v in reference.setup_inputs().items()}
    t0 = time.time()
    outs = kernel(**inputs)
    print("kernel done in", time.time() - t0, "s")
